# revision 59
# baseline (speedup 1.0000x reference)
"""Trainium2 Bass kernel for nn_AttentionBetweenWordsAndChars.

Reference computation (per batch b, word w):
    q/k/v projections of word_vec and char_vec (shared weights per proj),
    2x2 attention between the two representations, output [B, W, 2H].

Math used here (exact reformulation):
  - softmax over 2 keys == sigmoid of logit difference:
        attn[q, 0] = sigmoid((s_q,word - s_q,char))
  - out[q] = Vc + attn[q, 0] * (Vw - Vc) = Vc + sigmoid(d_q) * Dv
  - d0 = q_w . (k_w - k_c) = x~w A Dx^T,  d1 = x~c A Dx^T
    with A = W~q W~k^T / sqrt(H)  (301x301, built on device once),
    x~ = [x, 1] (bias-augmented), Dx = x~w - x~c.
  - Dv = Vw - Vc = Dx @ W~v  (bias rows cancel).
  So per 128-token tile only 3 matmul groups are needed (g = Dx@A^T, Dv,
  Vc) instead of 6 projections: ~2.6x less PE work, and K/Q never exist.

Sharding: data-parallel over batch; each of the 8 cores gets 8 batches
(4096 tokens). Weights replicated. No collectives.
"""

import sys

for _p in ("/opt/trn_rl_repo", "/root/.axon_site/_ro/trn_rl_repo"):
    if _p not in sys.path:
        sys.path.insert(0, _p)

import numpy as np

import concourse.bass as bass
import concourse.tile as tile
from concourse import mybir
from concourse.bass_utils import run_bass_kernel_spmd
import bass_rust

B, W, D_IN, H = 64, 512, 300, 512
N_CORES = 8
TOK = (B // N_CORES) * W          # 4096 tokens per core
TILES = TOK // 128                # 32
DA = D_IN + 1                     # 301 augmented dim
TEMP = float(np.sqrt(np.float32(H)))
F32 = mybir.dt.float32
BF16 = mybir.dt.bfloat16
AF = mybir.ActivationFunctionType
OP = mybir.AluOpType

# contraction-dim chunking of the 301 augmented features
CHUNKS = [(0, 128), (128, 128), (256, 45)]  # last = 44 features + bias row


def spill_excess_waits(nc, max_keep=1, ev_cap=2):
    """walrus accepts very few sync-wait commands per instruction (1 for
    most datapath opcodes). Move excess waits onto pure-wait EventSemaphore
    instructions inserted immediately before the offender on the same
    engine queue — semantically identical (FIFO queue), encoding-legal."""
    counter = 0
    for f in nc.m.functions:
        for blk in f.blocks:
            insts = blk.instructions
            i = 0
            while i < len(insts):
                inst = insts[i]
                si = inst.sync_info
                if si is None:
                    i += 1
                    continue
                w = list(si.on_wait or [])
                if len(w) > max_keep:
                    spill = w[:-max_keep]
                    for j in range(0, len(spill), ev_cap):
                        ev = mybir.InstEventSemaphore(name=f"wspill_{counter}")
                        counter += 1
                        ev.engine = inst.engine
                        ev.sync_info = bass_rust.SyncInfo(
                            on_wait=spill[j:j + ev_cap], on_update=[]
                        )
                        insts.insert(i, ev)
                        i += 1
                    inst.sync_info.on_wait = w[-max_keep:]
                i += 1
    return counter


PSUM_CFG = dict(g=3, dv=1, vc=1, t1=2, t2=1, wtp="ded")


def build_program(use_ttr=False, use_stt=True, use_gp_sub=True, loop_reps=1,
                  cfg=None):
    cfg = dict(PSUM_CFG, **(cfg or {}))
    nc = bass.Bass("TRN2", target_bir_lowering=False, debug=False,
                   num_devices=N_CORES)
    xw_d = nc.dram_tensor("word", [TOK, D_IN], BF16,
                          kind="ExternalInput").ap()
    xc_d = nc.dram_tensor("char", [TOK, D_IN], BF16,
                          kind="ExternalInput").ap()
    # host-prepacked bf16 [301, 1536] = [[Wq|Wk|Wv] ; [bq|bk|bv]]
    wcat_d = nc.dram_tensor("Wcat", [DA, 3 * H], BF16,
                            kind="ExternalInput").ap()
    out_d = nc.dram_tensor("out", [TOK, 2 * H], F32, kind="ExternalOutput").ap()
    import ml_dtypes as _mld
    eye_d = nc.inline_tensor(np.eye(128, dtype=_mld.bfloat16),
                             name="eye128").ap()

    from contextlib import ExitStack

    with tile.TileContext(nc) as tc, ExitStack() as es:
        cpool = es.enter_context(tc.tile_pool(name="consts", bufs=1))
        # PSUM pools are shared by the preamble and the main loop so that no
        # main-loop allocation ever waits on a preamble-pool release.
        ps_g = es.enter_context(
            tc.tile_pool(name="ps_g", bufs=cfg["g"], space="PSUM"))
        ps_dv = es.enter_context(
            tc.tile_pool(name="ps_dv", bufs=cfg["dv"], space="PSUM"))
        ps_vc = es.enter_context(
            tc.tile_pool(name="ps_vc", bufs=cfg["vc"], space="PSUM"))
        ps_t = es.enter_context(
            tc.tile_pool(name="ps_t", bufs=cfg["t1"], space="PSUM"))

        ident_b = cpool.tile([128, 128], BF16, tag="ident_b")
        nc.scalar.dma_start(ident_b[:], eye_d[:, :])
        zbias = cpool.tile([128, 1], F32, tag="zbias")
        nc.gpsimd.memset(zbias[:], 0.0)
        # preload the sigmoid ACT table set while the preamble runs
        sigwarm = cpool.tile([128, 1], F32, tag="sigwarm")
        nc.scalar.activation(sigwarm[:], zbias[:], AF.Sigmoid, bias=zbias[:])

        # resident bf16 weights: Wcat chunks stay resident; wv_b are slices
        stage = [cpool.tile([sz, 3 * H], BF16, tag=f"wcat{c}",
                            name=f"wcat{c}")
                 for c, (off, sz) in enumerate(CHUNKS)]
        last_wdma = None
        for c, (off, sz) in enumerate(CHUNKS):
            last_wdma = nc.scalar.dma_start(stage[c][:],
                                            wcat_d[off:off + sz, :])
        wv_b = [stage[c][:, 2 * H:3 * H] for c in range(3)]
        at_b = [cpool.tile([sz, DA], BF16, tag=f"at_b{c}", name=f"at_b{c}")
                for c, (off, sz) in enumerate(CHUNKS)]

        # ---------------- preamble: build A^T ----------------
        # transpose W~q, W~k (bf16 PE transpose). All three partition-chunks
        # of one h-slice share one PSUM bank so a single copy evacuates
        # [128, 301].
        wt = {}
        for nm in ("q", "k"):
            for h in range(4):
                wt[nm, h] = cpool.tile([128, DA], BF16, tag=f"wt{nm}{h}",
                                       name=f"wt{nm}{h}")
        # two (nm, h) groups per bf16 PSUM bank (cols 0:301 and 512:813)
        groups = [(nm, h) for nm in ("q", "k") for h in range(4)]
        wnames = {"q": 0, "k": 1}
        for gi in range(0, 8, 2):
            pt = ps_t.tile([128, 1024], BF16, tag="wtp", name="ptw", bufs=1)
            for slot, (nm, h) in enumerate(groups[gi:gi + 2]):
                wi = wnames[nm]
                base = slot * 512
                for c, (off, sz) in enumerate(CHUNKS):
                    nc.tensor.transpose(
                        pt[0:128, base + off:base + off + sz],
                        stage[c][:, wi * H + h * 128:
                                  wi * H + (h + 1) * 128],
                        ident_b[0:sz, 0:sz],
                    )
            for slot, (nm, h) in enumerate(groups[gi:gi + 2]):
                base = slot * 512
                nc.vector.tensor_copy(wt[nm, h][:],
                                      pt[0:128, base:base + DA])

        # A^T = W~k W~q^T / temp : 3 row-chunks x 4 h-chunks
        last_pre = None
        for m, (moff, msz) in enumerate(CHUNKS):
            ap = ps_g.tile([msz, DA], F32, tag="g", name="at_ps")
            for h in range(4):
                nc.tensor.matmul(
                    ap[:],
                    wt["k", h][:, moff:moff + msz],
                    wt["q", h][:],
                    start=(h == 0), stop=(h == 3),
                )
            last_pre = nc.scalar.mul(at_b[m][:], ap[:], 1.0 / TEMP)

        # ---------------- main loop over 32 token tiles ----------------
        px = es.enter_context(tc.tile_pool(name="px", bufs=6))
        pT = es.enter_context(tc.tile_pool(name="pT", bufs=6))
        psc = es.enter_context(tc.tile_pool(name="psc", bufs=6))
        pout = es.enter_context(tc.tile_pool(name="pout", bufs=6))

        for i in range(TILES * loop_reps):
            i = i % TILES
            r0 = i * 128
            xw = px.tile([128, 304], BF16, tag="xw")
            dma_w = nc.sync.dma_start(xw[:, 0:D_IN], xw_d[r0:r0 + 128, :])
            nc.gpsimd.memset(xw[:, D_IN:DA], 1.0)
            xc = px.tile([128, 304], BF16, tag="xc")
            dma_c = nc.sync.dma_start(xc[:, 0:D_IN], xc_d[r0:r0 + 128, :])
            nc.gpsimd.memset(xc[:, D_IN:DA], 1.0)
            if i < 3 and loop_reps == 1:
                # weight DMAs own the HBM bandwidth first; X loads are pure
                # prefetch until A^T is ready
                from concourse.tile import add_dep_helper
                add_dep_helper(dma_w.ins, last_wdma.ins,
                               reason="weights before X prefetch")
                add_dep_helper(dma_c.ins, last_wdma.ins,
                               reason="weights before X prefetch")

            # 6 fp32 transposes packed into 2 PSUM banks, then 3 copy+casts
            # to bf16 SBUF. The ones column rides along in chunk 2.
            t1 = ps_t.tile([128, 1024], BF16, tag="t1")
            trs = [
                nc.tensor.transpose(t1[0:128, 0:128], xw[:, 0:128],
                                    ident_b[:]),
                nc.tensor.transpose(t1[0:128, 128:256], xw[:, 128:256],
                                    ident_b[:]),
                nc.tensor.transpose(t1[0:128, 256:384], xc[:, 0:128],
                                    ident_b[:]),
                nc.tensor.transpose(t1[0:128, 384:512], xc[:, 128:256],
                                    ident_b[:]),
                nc.tensor.transpose(t1[0:45, 512:640], xw[:, 256:DA],
                                    ident_b[:]),
                nc.tensor.transpose(t1[0:45, 640:768], xc[:, 256:DA],
                                    ident_b[:]),
            ]
            if i < 2 and loop_reps == 1:
                # keep early iterations from starving the A^T preamble of
                # engine time (the whole loop depends on it)
                from concourse.tile import add_dep_helper
                for tr in trs:
                    add_dep_helper(tr.ins, last_pre.ins,
                                   reason="main loop waits for A^T build")

            xwT01 = pT.tile([128, 256], BF16, tag="xwT01")
            nc.scalar.copy(xwT01[:], t1[0:128, 0:256])
            xcT01 = pT.tile([128, 256], BF16, tag="xcT01")
            nc.scalar.copy(xcT01[:], t1[0:128, 256:512])
            xT2 = pT.tile([45, 256], BF16, tag="xT2")
            nc.scalar.copy(xT2[:], t1[0:45, 512:768])

            dx01 = pT.tile([128, 256], BF16, tag="dx01")
            dx2 = pT.tile([45, 128], BF16, tag="dx2")
            if use_gp_sub:
                nc.gpsimd.tensor_sub(dx01[:], xwT01[:], xcT01[:])
                nc.gpsimd.tensor_sub(dx2[:], xT2[0:45, 0:128],
                                     xT2[0:45, 128:256])
            else:
                nc.vector.tensor_sub(dx01[:], xwT01[:], xcT01[:])
                nc.vector.tensor_sub(dx2[:], xT2[0:45, 0:128],
                                     xT2[0:45, 128:256])
            dxT = [dx01[:, 0:128], dx01[:, 128:256], dx2[:]]
            xcT = [xcT01[:, 0:128], xcT01[:, 128:256], xT2[0:45, 128:256]]

            # matmuls: g = Dx A^T [128,301], Dv = Dx Wv, Vc = x~c Wv
            g_ps = ps_g.tile([128, DA], F32, tag="g")
            dv_ps = ps_dv.tile([128, H], F32, tag="dv")
            vc_ps = ps_vc.tile([128, H], F32, tag="vc")
            # vc first: it only needs the transpose copies, not the subs
            for c in range(3):
                nc.tensor.matmul(vc_ps[:], xcT[c], wv_b[c][:],
                                 start=(c == 0), stop=(c == 2))
            for c in range(3):
                st, sp = (c == 0), (c == 2)
                nc.tensor.matmul(g_ps[:], dxT[c], at_b[c][:],
                                 start=st, stop=sp)
                nc.tensor.matmul(dv_ps[:], dxT[c], wv_b[c][:],
                                 start=st, stop=sp)

            # logit differences d0, d1: one fused op each
            # (out = (g * 1.0) * x, accum_out = sum -> the dot product)
            dd = psc.tile([128, 2], F32, tag="dd")
            sc0 = psc.tile([128, DA], BF16, tag="sc0")
            nc.vector.scalar_tensor_tensor(
                out=sc0[:], in0=g_ps[:], scalar=1.0, in1=xw[:, 0:DA],
                op0=OP.mult, op1=OP.mult, accum_out=dd[:, 0:1])
            sc1 = psc.tile([128, DA], BF16, tag="sc1")
            nc.vector.scalar_tensor_tensor(
                out=sc1[:], in0=g_ps[:], scalar=1.0, in1=xc[:, 0:DA],
                op0=OP.mult, op1=OP.mult, accum_out=dd[:, 1:2])

            # attention weights
            aa = psc.tile([128, 2], F32, tag="aa")
            nc.scalar.activation(aa[:], dd[:], AF.Sigmoid, bias=zbias[:])

            # Dv to SBUF (ACT, fp32), then out_q = (Dv * a_q) + Vc  (DVE)
            dv_sb = psc.tile([128, H], F32, tag="dv_sb")
            nc.scalar.copy(dv_sb[:], dv_ps[:])
            out_t = pout.tile([128, 2 * H], F32, tag="out")
            if use_stt:
                nc.vector.scalar_tensor_tensor(
                    out=out_t[:, 0:H], in0=dv_sb[:], scalar=aa[:, 0:1],
                    in1=vc_ps[:], op0=OP.mult, op1=OP.add)
                nc.vector.scalar_tensor_tensor(
                    out=out_t[:, H:2 * H], in0=dv_sb[:], scalar=aa[:, 1:2],
                    in1=vc_ps[:], op0=OP.mult, op1=OP.add)
            else:
                sm0 = psc.tile([128, H], F32, tag="sm0")
                nc.vector.tensor_scalar_mul(sm0[:], dv_sb[:], aa[:, 0:1])
                nc.vector.tensor_add(out_t[:, 0:H], sm0[:], vc_ps[:])
                sm1 = psc.tile([128, H], F32, tag="sm1")
                nc.vector.tensor_scalar_mul(sm1[:], dv_sb[:], aa[:, 1:2])
                nc.vector.tensor_add(out_t[:, H:2 * H], sm1[:], vc_ps[:])

            nc.sync.dma_start(out_d[r0:r0 + 128, :], out_t[:])

    spill_excess_waits(nc)
    return nc


_CACHED = {}


def kernel(**inputs):
    if "nc" not in _CACHED:
        _CACHED["nc"] = build_program()
    nc = _CACHED["nc"]

    import ml_dtypes
    word = np.ascontiguousarray(
        np.asarray(inputs["word_vectors"]).astype(ml_dtypes.bfloat16))
    char = np.ascontiguousarray(
        np.asarray(inputs["char_vectors"]).astype(ml_dtypes.bfloat16))
    wcat = np.concatenate(
        [np.vstack([np.asarray(inputs[w], np.float32),
                    np.asarray(inputs[b], np.float32).reshape(1, H)])
         for w, b in (("Wq", "bq"), ("Wk", "bk"), ("Wv", "bv"))],
        axis=1,
    )
    base = {"Wcat": np.ascontiguousarray(wcat.astype(ml_dtypes.bfloat16))}
    bpc = B // N_CORES
    in_maps = []
    for c in range(N_CORES):
        m = dict(base)
        m["word"] = word[c * bpc:(c + 1) * bpc].reshape(TOK, D_IN)
        m["char"] = char[c * bpc:(c + 1) * bpc].reshape(TOK, D_IN)
        in_maps.append(m)

    res = run_bass_kernel_spmd(nc, in_maps, list(range(N_CORES)))
    out = np.concatenate(
        [res.results[c]["out"].reshape(bpc, W, 2 * H) for c in range(N_CORES)],
        axis=0,
    )
    return out


# revision 64
# speedup vs baseline: 1.0333x; 1.0333x over previous
"""Trainium2 Bass kernel for nn_AttentionBetweenWordsAndChars.

Reference computation (per batch b, word w):
    q/k/v projections of word_vec and char_vec (shared weights per proj),
    2x2 attention between the two representations, output [B, W, 2H].

Math used here (exact reformulation):
  - softmax over 2 keys == sigmoid of logit difference:
        attn[q, 0] = sigmoid((s_q,word - s_q,char))
  - out[q] = Vc + attn[q, 0] * (Vw - Vc) = Vc + sigmoid(d_q) * Dv
  - d0 = q_w . (k_w - k_c) = x~w A Dx^T,  d1 = x~c A Dx^T
    with A = W~q W~k^T / sqrt(H)  (301x301, built on device once),
    x~ = [x, 1] (bias-augmented), Dx = x~w - x~c.
  - Dv = Vw - Vc = Dx @ W~v  (bias rows cancel).
  So per 128-token tile only 3 matmul groups are needed (g = Dx@A^T, Dv,
  Vc) instead of 6 projections: ~2.6x less PE work, and K/Q never exist.

Sharding: data-parallel over batch; each of the 8 cores gets 8 batches
(4096 tokens). Weights replicated. No collectives.
"""

import sys

for _p in ("/opt/trn_rl_repo", "/root/.axon_site/_ro/trn_rl_repo"):
    if _p not in sys.path:
        sys.path.insert(0, _p)

import numpy as np

import concourse.bass as bass
import concourse.tile as tile
from concourse import mybir
from concourse.bass_utils import run_bass_kernel_spmd
import bass_rust

B, W, D_IN, H = 64, 512, 300, 512
N_CORES = 8
TOK = (B // N_CORES) * W          # 4096 tokens per core
TILES = TOK // 128                # 32
DA = D_IN + 1                     # 301 augmented dim
TEMP = float(np.sqrt(np.float32(H)))
F32 = mybir.dt.float32
BF16 = mybir.dt.bfloat16
AF = mybir.ActivationFunctionType
OP = mybir.AluOpType

# contraction-dim chunking of the 301 augmented features
CHUNKS = [(0, 128), (128, 128), (256, 45)]  # last = 44 features + bias row


def spill_excess_waits(nc, max_keep=1, ev_cap=2):
    """walrus accepts very few sync-wait commands per instruction (1 for
    most datapath opcodes). Move excess waits onto pure-wait EventSemaphore
    instructions inserted immediately before the offender on the same
    engine queue — semantically identical (FIFO queue), encoding-legal."""
    counter = 0
    for f in nc.m.functions:
        for blk in f.blocks:
            insts = blk.instructions
            i = 0
            while i < len(insts):
                inst = insts[i]
                si = inst.sync_info
                if si is None:
                    i += 1
                    continue
                w = list(si.on_wait or [])
                if len(w) > max_keep:
                    spill = w[:-max_keep]
                    for j in range(0, len(spill), ev_cap):
                        ev = mybir.InstEventSemaphore(name=f"wspill_{counter}")
                        counter += 1
                        ev.engine = inst.engine
                        ev.sync_info = bass_rust.SyncInfo(
                            on_wait=spill[j:j + ev_cap], on_update=[]
                        )
                        insts.insert(i, ev)
                        i += 1
                    inst.sync_info.on_wait = w[-max_keep:]
                i += 1
    return counter


PSUM_CFG = dict(g=3, dv=1, vc=1, t1=2, t2=1, wtp="t1")


def build_program(use_ttr=False, use_stt=True, use_gp_sub=True, loop_reps=1,
                  cfg=None):
    cfg = dict(PSUM_CFG, **(cfg or {}))
    nc = bass.Bass("TRN2", target_bir_lowering=False, debug=False,
                   num_devices=N_CORES)
    xw_d = nc.dram_tensor("word", [TOK, D_IN], BF16,
                          kind="ExternalInput").ap()
    xc_d = nc.dram_tensor("char", [TOK, D_IN], BF16,
                          kind="ExternalInput").ap()
    # host-prepacked bf16 [301, 1536] = [[Wq|Wk|Wv] ; [bq|bk|bv]]
    wcat_d = nc.dram_tensor("Wcat", [DA, 3 * H], BF16,
                            kind="ExternalInput").ap()
    out_d = nc.dram_tensor("out", [TOK, 2 * H], F32, kind="ExternalOutput").ap()
    import ml_dtypes as _mld
    eye_d = nc.inline_tensor(np.eye(128, dtype=_mld.bfloat16),
                             name="eye128").ap()

    from contextlib import ExitStack

    with tile.TileContext(nc) as tc, ExitStack() as es:
        cpool = es.enter_context(tc.tile_pool(name="consts", bufs=1))
        # PSUM pools are shared by the preamble and the main loop so that no
        # main-loop allocation ever waits on a preamble-pool release.
        ps_g = es.enter_context(
            tc.tile_pool(name="ps_g", bufs=cfg["g"], space="PSUM"))
        ps_dv = es.enter_context(
            tc.tile_pool(name="ps_dv", bufs=cfg["dv"], space="PSUM"))
        ps_vc = es.enter_context(
            tc.tile_pool(name="ps_vc", bufs=cfg["vc"], space="PSUM"))
        ps_t = es.enter_context(
            tc.tile_pool(name="ps_t", bufs=cfg["t1"], space="PSUM"))

        ident_b = cpool.tile([128, 128], BF16, tag="ident_b")
        nc.scalar.dma_start(ident_b[:], eye_d[:, :])
        zbias = cpool.tile([128, 1], F32, tag="zbias")
        nc.gpsimd.memset(zbias[:], 0.0)
        # preload the sigmoid ACT table set while the preamble runs
        sigwarm = cpool.tile([128, 1], F32, tag="sigwarm")
        nc.scalar.activation(sigwarm[:], zbias[:], AF.Sigmoid, bias=zbias[:])

        # resident bf16 weights: Wcat chunks stay resident; wv_b are slices
        stage = [cpool.tile([sz, 3 * H], BF16, tag=f"wcat{c}",
                            name=f"wcat{c}")
                 for c, (off, sz) in enumerate(CHUNKS)]
        last_wdma = None
        for c, (off, sz) in enumerate(CHUNKS):
            last_wdma = nc.scalar.dma_start(stage[c][:],
                                            wcat_d[off:off + sz, :])
        wv_b = [stage[c][:, 2 * H:3 * H] for c in range(3)]
        at_b = [cpool.tile([sz, DA], BF16, tag=f"at_b{c}", name=f"at_b{c}")
                for c, (off, sz) in enumerate(CHUNKS)]

        # ---------------- preamble: build A^T ----------------
        # transpose W~q, W~k (bf16 PE transpose). All three partition-chunks
        # of one h-slice share one PSUM bank so a single copy evacuates
        # [128, 301].
        wt = {}
        for nm in ("q", "k"):
            for h in range(4):
                wt[nm, h] = cpool.tile([128, DA], BF16, tag=f"wt{nm}{h}",
                                       name=f"wt{nm}{h}")
        # two (nm, h) groups per bf16 PSUM bank (cols 0:301 and 512:813)
        groups = [(nm, h) for nm in ("q", "k") for h in range(4)]
        wnames = {"q": 0, "k": 1}
        for gi in range(0, 8, 2):
            if cfg["wtp"] == "t1":
                pt = ps_t.tile([128, 1024], BF16, tag="t1", name="ptw")
            else:
                pt = ps_t.tile([128, 1024], BF16, tag="wtp", name="ptw",
                               bufs=1)
            for slot, (nm, h) in enumerate(groups[gi:gi + 2]):
                wi = wnames[nm]
                base = slot * 512
                for c, (off, sz) in enumerate(CHUNKS):
                    nc.tensor.transpose(
                        pt[0:128, base + off:base + off + sz],
                        stage[c][:, wi * H + h * 128:
                                  wi * H + (h + 1) * 128],
                        ident_b[0:sz, 0:sz],
                    )
            for slot, (nm, h) in enumerate(groups[gi:gi + 2]):
                base = slot * 512
                nc.vector.tensor_copy(wt[nm, h][:],
                                      pt[0:128, base:base + DA])

        # A^T = W~k W~q^T / temp : 3 row-chunks x 4 h-chunks
        last_pre = None
        for m, (moff, msz) in enumerate(CHUNKS):
            ap = ps_g.tile([msz, DA], F32, tag="g", name="at_ps")
            for h in range(4):
                nc.tensor.matmul(
                    ap[:],
                    wt["k", h][:, moff:moff + msz],
                    wt["q", h][:],
                    start=(h == 0), stop=(h == 3),
                )
            last_pre = nc.scalar.mul(at_b[m][:], ap[:], 1.0 / TEMP)

        # ---------------- main loop over 32 token tiles ----------------
        px = es.enter_context(tc.tile_pool(name="px", bufs=6))
        pT = es.enter_context(tc.tile_pool(name="pT", bufs=6))
        psc = es.enter_context(tc.tile_pool(name="psc", bufs=6))
        pout = es.enter_context(tc.tile_pool(name="pout", bufs=6))

        def emit_combine(dv_sb, vc_ps, aa, r0):
            out_t = pout.tile([128, 2 * H], F32, tag="out", name="out_t")
            nc.vector.scalar_tensor_tensor(
                out=out_t[:, 0:H], in0=dv_sb[:], scalar=aa[:, 0:1],
                in1=vc_ps[:], op0=OP.mult, op1=OP.add)
            nc.vector.scalar_tensor_tensor(
                out=out_t[:, H:2 * H], in0=dv_sb[:], scalar=aa[:, 1:2],
                in1=vc_ps[:], op0=OP.mult, op1=OP.add)
            nc.sync.dma_start(out_d[r0:r0 + 128, :], out_t[:])

        for i in range(TILES * loop_reps):
            i = i % TILES
            r0 = i * 128
            xw = px.tile([128, 304], BF16, tag="xw")
            dma_w = nc.sync.dma_start(xw[:, 0:D_IN], xw_d[r0:r0 + 128, :])
            nc.gpsimd.memset(xw[:, D_IN:DA], 1.0)
            xc = px.tile([128, 304], BF16, tag="xc")
            dma_c = nc.sync.dma_start(xc[:, 0:D_IN], xc_d[r0:r0 + 128, :])
            nc.gpsimd.memset(xc[:, D_IN:DA], 1.0)
            if i < 3 and loop_reps == 1:
                # weight DMAs own the HBM bandwidth first; X loads are pure
                # prefetch until A^T is ready
                from concourse.tile import add_dep_helper
                add_dep_helper(dma_w.ins, last_wdma.ins,
                               reason="weights before X prefetch")
                add_dep_helper(dma_c.ins, last_wdma.ins,
                               reason="weights before X prefetch")

            # 6 fp32 transposes packed into 2 PSUM banks, then 3 copy+casts
            # to bf16 SBUF. The ones column rides along in chunk 2.
            t1 = ps_t.tile([128, 1024], BF16, tag="t1")
            trs = [
                nc.tensor.transpose(t1[0:128, 0:128], xw[:, 0:128],
                                    ident_b[:]),
                nc.tensor.transpose(t1[0:128, 128:256], xw[:, 128:256],
                                    ident_b[:]),
                nc.tensor.transpose(t1[0:128, 256:384], xc[:, 0:128],
                                    ident_b[:]),
                nc.tensor.transpose(t1[0:128, 384:512], xc[:, 128:256],
                                    ident_b[:]),
                nc.tensor.transpose(t1[0:45, 512:640], xw[:, 256:DA],
                                    ident_b[:]),
                nc.tensor.transpose(t1[0:45, 640:768], xc[:, 256:DA],
                                    ident_b[:]),
            ]
            if i < 2 and loop_reps == 1:
                # keep early iterations from starving the A^T preamble of
                # engine time (the whole loop depends on it)
                from concourse.tile import add_dep_helper
                for tr in trs:
                    add_dep_helper(tr.ins, last_pre.ins,
                                   reason="main loop waits for A^T build")

            xwT01 = pT.tile([128, 256], BF16, tag="xwT01")
            nc.scalar.copy(xwT01[:], t1[0:128, 0:256])
            xcT01 = pT.tile([128, 256], BF16, tag="xcT01")
            nc.scalar.copy(xcT01[:], t1[0:128, 256:512])
            xT2 = pT.tile([45, 256], BF16, tag="xT2")
            nc.scalar.copy(xT2[:], t1[0:45, 512:768])

            dx01 = pT.tile([128, 256], BF16, tag="dx01")
            dx2 = pT.tile([45, 128], BF16, tag="dx2")
            if use_gp_sub:
                nc.gpsimd.tensor_sub(dx01[:], xwT01[:], xcT01[:])
                nc.gpsimd.tensor_sub(dx2[:], xT2[0:45, 0:128],
                                     xT2[0:45, 128:256])
            else:
                nc.vector.tensor_sub(dx01[:], xwT01[:], xcT01[:])
                nc.vector.tensor_sub(dx2[:], xT2[0:45, 0:128],
                                     xT2[0:45, 128:256])
            dxT = [dx01[:, 0:128], dx01[:, 128:256], dx2[:]]
            xcT = [xcT01[:, 0:128], xcT01[:, 128:256], xT2[0:45, 128:256]]

            # matmuls: g = Dx A^T [128,301], Dv = Dx Wv, Vc = x~c Wv
            g_ps = ps_g.tile([128, DA], F32, tag="g")
            dv_ps = ps_dv.tile([128, H], F32, tag="dv")
            vc_ps = ps_vc.tile([128, H], F32, tag="vc")
            # vc first: it only needs the transpose copies, not the subs
            for c in range(3):
                nc.tensor.matmul(vc_ps[:], xcT[c], wv_b[c][:],
                                 start=(c == 0), stop=(c == 2))
            for c in range(3):
                st, sp = (c == 0), (c == 2)
                nc.tensor.matmul(g_ps[:], dxT[c], at_b[c][:],
                                 start=st, stop=sp)
                nc.tensor.matmul(dv_ps[:], dxT[c], wv_b[c][:],
                                 start=st, stop=sp)

            # logit differences d0, d1: one fused op each
            # (out = (g * 1.0) * x, accum_out = sum -> the dot product)
            dd = psc.tile([128, 2], F32, tag="dd")
            sc0 = psc.tile([128, DA], BF16, tag="sc0")
            nc.vector.scalar_tensor_tensor(
                out=sc0[:], in0=g_ps[:], scalar=1.0, in1=xw[:, 0:DA],
                op0=OP.mult, op1=OP.mult, accum_out=dd[:, 0:1])
            sc1 = psc.tile([128, DA], BF16, tag="sc1")
            nc.vector.scalar_tensor_tensor(
                out=sc1[:], in0=g_ps[:], scalar=1.0, in1=xc[:, 0:DA],
                op0=OP.mult, op1=OP.mult, accum_out=dd[:, 1:2])

            # attention weights
            # Dv to SBUF (ACT, fp32) before the sigmoid: the copy's input
            # is ready earlier, so it must not queue behind a stalled sigmoid
            dv_sb = psc.tile([128, H], F32, tag="dv_sb")
            nc.scalar.copy(dv_sb[:], dv_ps[:])
            aa = psc.tile([128, 2], F32, tag="aa")
            nc.scalar.activation(aa[:], dd[:], AF.Sigmoid, bias=zbias[:])

            emit_combine(dv_sb, vc_ps, aa, r0)

    spill_excess_waits(nc)
    return nc


_CACHED = {}


def kernel(**inputs):
    if "nc" not in _CACHED:
        _CACHED["nc"] = build_program()
    nc = _CACHED["nc"]

    import ml_dtypes
    word = np.ascontiguousarray(
        np.asarray(inputs["word_vectors"]).astype(ml_dtypes.bfloat16))
    char = np.ascontiguousarray(
        np.asarray(inputs["char_vectors"]).astype(ml_dtypes.bfloat16))
    wcat = np.concatenate(
        [np.vstack([np.asarray(inputs[w], np.float32),
                    np.asarray(inputs[b], np.float32).reshape(1, H)])
         for w, b in (("Wq", "bq"), ("Wk", "bk"), ("Wv", "bv"))],
        axis=1,
    )
    base = {"Wcat": np.ascontiguousarray(wcat.astype(ml_dtypes.bfloat16))}
    bpc = B // N_CORES
    in_maps = []
    for c in range(N_CORES):
        m = dict(base)
        m["word"] = word[c * bpc:(c + 1) * bpc].reshape(TOK, D_IN)
        m["char"] = char[c * bpc:(c + 1) * bpc].reshape(TOK, D_IN)
        in_maps.append(m)

    res = run_bass_kernel_spmd(nc, in_maps, list(range(N_CORES)))
    out = np.concatenate(
        [res.results[c]["out"].reshape(bpc, W, 2 * H) for c in range(N_CORES)],
        axis=0,
    )
    return out
